# revision 6
# baseline (speedup 1.0000x reference)
"""ROIPooler (FPN ROIAlign, P=7, ratio=2) Trainium2 Bass kernel.

Strategy (8 NeuronCores, data-parallel over ROIs):
- 1024 ROIs -> 128 per core; cores 0-3 work on image 0, cores 4-7 on image 1.
- Host precomputes, per ROI, its FPN level, the 196 bilinear sample windows
  (49 bins x 4 subsamples) and the 4 corner weights per window (validity,
  edge clamps and the 1/4 average folded in).
- Each image's feature pyramid is laid out host-side as a "row-pair" arena:
  entry (l, y, x) = [feat_l[y, x, :C], feat_l[y+1, x, :C]] fp32, so ONE
  contiguous 1024-element read (2 entries) covers all 4 bilinear corners
  of a subsample.
- Device: per K-tile of 128 refs, one indirect DMA gathers [128, 1024];
  4 masked matmuls (lhsT = static one-hot bin mask x per-ref corner weight,
  built on DVE) reduce refs into PSUM bins; PSUM groups of 128 bins are
  evacuated and DMA'd to a dense [6272, 256] output per core.
"""
import numpy as np

P_OUT = 7
RATIO = 2
SCALES = (0.25, 0.125, 0.0625, 0.03125)
LEVEL_DIMS = ((200, 336), (100, 168), (50, 84), (25, 42))
CANONICAL_BOX_SIZE = 224.0
CANONICAL_LEVEL = 4.0
EPS = 2.220446049250313e-16
C = 256
ENTRY = 2 * C                 # row-pair entry elems
WIN = 2 * ENTRY               # gather window elems (2 entries = 4 corners)
BINS = 49
REFS_PER_BIN = 4              # 2x2 subsamples
REFS_PER_ROI = BINS * REFS_PER_BIN       # 196
ROIS_PER_CORE = 128
N_CORES = 8
N_TILES = ROIS_PER_CORE * REFS_PER_ROI // 128    # 196 K-tiles per core
N_BINS_CORE = ROIS_PER_CORE * BINS               # 6272 = 49 groups of 128
T_CHUNK = 12                  # K-tiles per chunk (3 psum groups)
PLANE_PX = [h * w for h, w in LEVEL_DIMS]
PLANE_BASE_PX = [0]
for _px in PLANE_PX:
    PLANE_BASE_PX.append(PLANE_BASE_PX[-1] + _px)
ARENA_PX = PLANE_BASE_PX[-1]                     # 89250 entries
ARENA_ELEMS = ARENA_PX * ENTRY + WIN             # + pad tail

_CHUNKS = []
_t = 0
while _t < N_TILES:
    _CHUNKS.append(min(T_CHUNK, N_TILES - _t))
    _t += T_CHUNK
N_CHUNKS = len(_CHUNKS)                          # 17 (16x12 + 1x4)
PAR_COLS = 5 * T_CHUNK                           # idx + 4 weights per tile slot


GATHER_DTYPE = np.float32


def _build_arena(x2, x3, x4, x5, b):
    parts = []
    for x in (x2, x3, x4, x5):
        hwc = np.ascontiguousarray(np.transpose(x[b], (1, 2, 0))).astype(GATHER_DTYPE)
        H, W, _ = hwc.shape
        rp = np.zeros((H, W, 2, C), GATHER_DTYPE)
        rp[:, :, 0] = hwc
        rp[:-1, :, 1] = hwc[1:]
        parts.append(rp.reshape(-1))
    parts.append(np.zeros(WIN, GATHER_DTYPE))
    return np.concatenate(parts)


def _compute_refs(boxes):
    """-> idx [N, 196] int32 (arena element offsets), w4 [N, 196, 4] f32.
    Ref order per ROI: bin-major (by*7+bx), then j = sy*2 + sx.
    Window corners: [0:256]=(y,x) [256:512]=(y+1,x) [512:768]=(y,x+1) [768:1024]=(y+1,x+1)."""
    f32 = np.float32
    B, R, _ = boxes.shape
    N = B * R
    flat = boxes.reshape(N, 4).astype(f32)
    x1, y1, x2b, y2b = flat[:, 0], flat[:, 1], flat[:, 2], flat[:, 3]

    area = (x2b - x1) * (y2b - y1)
    sizes = np.sqrt(area, dtype=f32)
    lvl_f = np.floor(f32(CANONICAL_LEVEL)
                     + np.log2(sizes / f32(CANONICAL_BOX_SIZE) + f32(EPS), dtype=f32))
    lvl = np.clip(lvl_f, 2, 5).astype(np.int32) - 2

    scale = np.array(SCALES, f32)[lvl]
    H = np.array([d[0] for d in LEVEL_DIMS], np.int32)[lvl]
    W = np.array([d[1] for d in LEVEL_DIMS], np.int32)[lvl]
    base = np.array(PLANE_BASE_PX, np.int64)[:4][lvl]

    x1s = x1 * scale
    y1s = y1 * scale
    roi_w = np.maximum(x2b * scale - x1s, f32(1.0))
    roi_h = np.maximum(y2b * scale - y1s, f32(1.0))

    g = RATIO
    off = (np.arange(P_OUT, dtype=f32)[:, None]
           + (np.arange(g, dtype=f32)[None, :] + f32(0.5)) / f32(g)).reshape(-1)
    ys = y1s[:, None] + off[None, :] * (roi_h / f32(P_OUT))[:, None]
    xs = x1s[:, None] + off[None, :] * (roi_w / f32(P_OUT))[:, None]
    my = (ys >= -1.0) & (ys <= H[:, None])
    mx = (xs >= -1.0) & (xs <= W[:, None])

    def prep(t, L):
        t = np.maximum(t, f32(0.0))
        lo = np.floor(t).astype(np.int32)
        over = lo >= (L[:, None] - 1)
        lo = np.where(over, L[:, None] - 1, lo)
        t = np.where(over, lo.astype(f32), t)
        return lo, (t - lo).astype(f32)

    yl, fy = prep(ys, H)
    xl, fx = prep(xs, W)

    valid = (my[:, :, None] & mx[:, None, :]).astype(f32)     # [N,14,14]
    quarter = f32(1.0 / (g * g))

    YL = yl.reshape(N, P_OUT, g, 1, 1)
    FY = fy.reshape(N, P_OUT, g, 1, 1)
    XL = xl.reshape(N, 1, 1, P_OUT, g)
    FX = fx.reshape(N, 1, 1, P_OUT, g)
    V = valid.reshape(N, P_OUT, g, P_OUT, g)
    Wl = W.reshape(N, 1, 1, 1, 1).astype(np.int64)
    basel = base.reshape(N, 1, 1, 1, 1)

    ent = basel + YL.astype(np.int64) * Wl + XL.astype(np.int64)    # entry idx
    w00 = (1 - FY) * (1 - FX) * V * quarter
    w10 = FY * (1 - FX) * V * quarter
    w01 = (1 - FY) * FX * V * quarter
    w11 = FY * FX * V * quarter
    w4 = np.stack([w00, w10, w01, w11], axis=-1)              # [N,7,2,7,2,4]

    # (by, sy, bx, sx) -> (by, bx, sy, sx)
    ent = ent.transpose(0, 1, 3, 2, 4).reshape(N, REFS_PER_ROI)
    w4 = w4.transpose(0, 1, 3, 2, 4, 5).reshape(N, REFS_PER_ROI, 4).astype(f32)
    idx = (ent * ENTRY).astype(np.int32)
    return idx, w4


def _pack_core(idx_core, w4_core):
    """idx_core [128, 196], w4_core [128, 196, 4] -> par [N_CHUNKS, 128, PAR_COLS] int32.
    K-tile (q, r, h): partition p -> bin gb = 128q + 64r + (p//2), ref j = 2h + (p%2)."""
    idx_flat = idx_core.reshape(-1, REFS_PER_BIN)             # [6272 bins, 4]
    w_flat = w4_core.reshape(-1, REFS_PER_BIN, 4)
    tiles_idx = np.zeros((N_TILES, 128), np.int32)
    tiles_w = np.zeros((N_TILES, 4, 128), np.float32)
    p = np.arange(128)
    m = p // 2
    b2 = p % 2
    for q in range(N_BINS_CORE // 128):
        for r in range(2):
            for h in range(2):
                t = q * 4 + r * 2 + h
                gb = 128 * q + 64 * r + m
                j = 2 * h + b2
                tiles_idx[t] = idx_flat[gb, j]
                tiles_w[t] = w_flat[gb, j].T                  # [4, 128]
    par = np.zeros((N_CHUNKS, 128, PAR_COLS), np.int32)
    for c, tc in enumerate(_CHUNKS):
        t0 = c * T_CHUNK
        blk_i = tiles_idx[t0:t0 + tc]                         # [tc, 128]
        blk_w = tiles_w[t0:t0 + tc]                           # [tc, 4, 128]
        par[c, :, 0:tc] = blk_i.T
        for q4 in range(4):
            par[c, :, (1 + q4) * T_CHUNK:(1 + q4) * T_CHUNK + tc] = \
                blk_w[:, q4, :].T.view(np.int32)
    return par


_PROGRAM = None
TRACE = False
LAST_EXEC_NS = None


def _enable_ntff_hook():
    """Register the NTFF profile hook missing from this image's antenv."""
    import sys as _sys
    import types as _types
    try:
        from antenv.axon_hooks import get_axon_ntff_profile_hook  # noqa: F401
        return
    except ImportError:
        pass
    try:
        import trn_agent_boot.trn_boot as _tb
        _mod = _types.ModuleType("antenv.axon_hooks")
        _state = {"hook": _tb._ntff_profile_via_ctypes("/opt/axon/libaxon_pjrt.so")}
        _mod.set_axon_ntff_profile_hook = lambda h: _state.__setitem__("hook", h)
        _mod.get_axon_ntff_profile_hook = lambda: _state["hook"]
        _sys.modules["antenv.axon_hooks"] = _mod
    except Exception:
        pass


def _build_program():
    import concourse.bass as bass
    from concourse import mybir

    nc = bass.Bass()
    arena_ext = nc.declare_dram_parameter("arena", [ARENA_ELEMS], mybir.dt.float32, isOutput=False)
    par_ext = nc.declare_dram_parameter("par", [N_CHUNKS, 128, PAR_COLS], mybir.dt.int32, isOutput=False)
    mask_ext = nc.declare_dram_parameter("mask2", [128, 64], mybir.dt.float32, isOutput=False)
    out_ext = nc.declare_dram_parameter("out", [N_BINS_CORE, C], mybir.dt.float32, isOutput=True)

    T = T_CHUNK
    n_groups = [tc // 4 for tc in _CHUNKS]        # psum groups per chunk
    gd = np.cumsum([0] + n_groups)                # groups done after chunk c-1
    td = np.cumsum([0] + _CHUNKS)                 # tiles done
    NG = int(gd[-1])                              # 49

    with (
        nc.sbuf_tensor([128, 2 * PAR_COLS], mybir.dt.int32) as par_t,
        nc.sbuf_tensor([128, 64], mybir.dt.float32) as mask_t,
        nc.sbuf_tensor([128, 2 * T * 4 * 64], mybir.dt.float32) as W_t,
        nc.sbuf_tensor([128, 2 * T * WIN], mybir.dt.float32) as G_t,
        nc.sbuf_tensor([128, 2 * 3 * C], mybir.dt.float32) as stg_t,
        nc.psum_tensor([128, 4096], mybir.dt.float32) as ps,
        nc.semaphore("par_sem0") as par_sem0,
        nc.semaphore("par_sem1") as par_sem1,
        nc.semaphore("mask_sem") as mask_sem,
        nc.semaphore("gather_sem0") as gather_sem0,
        nc.semaphore("gather_sem1") as gather_sem1,
        nc.semaphore("w_sem") as w_sem,
        nc.semaphore("mm_sem") as mm_sem,
        nc.semaphore("evac_sem") as evac_sem,
        nc.semaphore("od_sem0") as od_sem0,
        nc.semaphore("od_sem1") as od_sem1,
        nc.Block() as block,
    ):
        par_sems = (par_sem0, par_sem1)
        gather_sems = (gather_sem0, gather_sem1)
        od_sems = (od_sem0, od_sem1)

        def par_count(cc):
            # value of par_sems[cc % 2] after par DMA of chunk cc completes
            return 16 * (cc // 2 + 1)

        def gather_count(cc):
            # value of gather_sems[cc % 2] after all gathers of chunk cc complete
            return 16 * sum(tcj for j, tcj in enumerate(_CHUNKS)
                            if j <= cc and j % 2 == cc % 2)

        def od_count(cc):
            # value of od_sems[cc % 2] after out-DMA of chunk cc completes
            return 16 * (cc // 2 + 1)

        def par_buf(c):
            return par_t[:, (c % 2) * PAR_COLS:(c % 2 + 1) * PAR_COLS]

        def w_buf(c):
            off = (c % 2) * T * 4 * 64
            return W_t[:, off:off + T * 4 * 64]

        def g_buf(c):
            off = (c % 2) * T * WIN
            return G_t[:, off:off + T * WIN]

        def stg_buf(c):
            off = (c % 2) * 3 * C
            return stg_t[:, off:off + 3 * C]

        @block.sync
        def _(sync):
            sync.dma_start(out=mask_t[:], in_=mask_ext[:]).then_inc(mask_sem, 16)

            def out_dma(sync, k):
                sync.wait_ge(evac_sem, int(gd[k + 1]))
                ng = n_groups[k]
                dst = out_ext[:].rearrange("(a p) c -> p a c", p=128)[
                    :, 3 * k:3 * k + ng, :]
                sync.dma_start(out=dst, in_=stg_buf(k)[:, :ng * C]).then_inc(
                    od_sems[k % 2], 16)

            for c in range(N_CHUNKS):
                if c >= 2:
                    # par buffer reuse: gather + W-build of c-2 finished
                    sync.wait_ge(gather_sems[c % 2], gather_count(c - 2))
                    sync.wait_ge(w_sem, c - 1)
                sync.dma_start(out=par_buf(c), in_=par_ext[c]).then_inc(
                    par_sems[c % 2], 16)
                if c >= 2:
                    out_dma(sync, c - 2)
            for k in (N_CHUNKS - 2, N_CHUNKS - 1):
                out_dma(sync, k)

        @block.gpsimd
        def _(g):
            for c, tc in enumerate(_CHUNKS):
                g.wait_ge(par_sems[c % 2], par_count(c))
                if c >= 2:
                    g.wait_ge(mm_sem, int(gd[c - 1]))   # G buffer reuse
                idxs = par_buf(c)
                gb = g_buf(c)
                for t in range(tc):
                    g.indirect_dma_start(
                        out=gb[:, t * WIN:(t + 1) * WIN],
                        out_offset=None,
                        in_=arena_ext[:].rearrange("(a b) -> a b", a=1),
                        in_offset=bass.IndirectOffsetOnAxis(ap=idxs[:, t:t + 1], axis=1),
                    ).then_inc(gather_sems[c % 2], 16)

        def _evacs(vector, k):
            ng = n_groups[k]
            if k >= 2:
                vector.wait_ge(od_sems[k % 2], od_count(k - 2))  # staging reuse
            for kg in range(ng):
                vector.wait_ge(mm_sem, int(gd[k]) + kg + 1)
                vector.tensor_copy(
                    out=stg_buf(k)[:, kg * C:(kg + 1) * C],
                    in_=ps[:, ((k % 2) * 4 + kg) * 512:((k % 2) * 4 + kg) * 512 + C],
                ).then_inc(evac_sem, 1)

        @block.vector
        def _(vector):
            vector.wait_ge(mask_sem, 16)
            for c, tc in enumerate(_CHUNKS):
                vector.wait_ge(par_sems[c % 2], par_count(c))
                if c >= 2:
                    vector.wait_ge(mm_sem, int(gd[c - 1]))  # W buffer reuse
                wsrc = par_buf(c)[:, T:5 * T].bitcast(mybir.dt.float32)
                # W[p, (t*4+q)*64 + m] = mask2[p, m] * w[p, q, t]
                vector.tensor_tensor(
                    out=w_buf(c)[:, :tc * 4 * 64].rearrange(
                        "p (t q m) -> p t q m", q=4, m=64),
                    in0=mask_t[:].rearrange("p (t q m) -> p t q m", t=1, q=1)
                        .to_broadcast([128, tc, 4, 64]),
                    in1=wsrc.rearrange("p (q t) -> p t q", q=4)[:, 0:tc, :]
                        .rearrange("p t (q m) -> p t q m", m=1)
                        .to_broadcast([128, tc, 4, 64]),
                    op=mybir.AluOpType.mult,
                ).then_inc(w_sem, 1)
                if c >= 1:
                    _evacs(vector, c - 1)
            _evacs(vector, N_CHUNKS - 1)

        @block.tensor
        def _(tensor):
            for c, tc in enumerate(_CHUNKS):
                tensor.wait_ge(gather_sems[c % 2], gather_count(c))
                tensor.wait_ge(w_sem, c + 1)
                if c >= 2:
                    tensor.wait_ge(evac_sem, int(gd[c - 1]))  # psum reuse
                wb = w_buf(c)
                gb = g_buf(c)
                for kg in range(tc // 4):
                    pcol = ((c % 2) * 4 + kg) * 512
                    for r in range(2):
                        rows = slice(64 * r, 64 * r + 64)
                        for h in range(2):
                            t = kg * 4 + r * 2 + h
                            for q4 in range(4):
                                mm = tensor.matmul(
                                    out=ps[rows, pcol:pcol + C],
                                    lhsT=wb[:, (t * 4 + q4) * 64:(t * 4 + q4 + 1) * 64],
                                    rhs=gb[:, t * WIN + q4 * C:t * WIN + (q4 + 1) * C],
                                    start=(h == 0 and q4 == 0),
                                    stop=(h == 1 and q4 == 3),
                                )
                                if r == 1 and h == 1 and q4 == 3:
                                    mm.then_inc(mm_sem, 1)
    return nc


def _get_program():
    global _PROGRAM
    if _PROGRAM is None:
        _PROGRAM = _build_program()
    return _PROGRAM


def kernel(x2, x3, x4, x5, boxes):
    from concourse.bass_utils import run_bass_kernel_spmd

    x2 = np.asarray(x2, np.float32)
    x3 = np.asarray(x3, np.float32)
    x4 = np.asarray(x4, np.float32)
    x5 = np.asarray(x5, np.float32)
    boxes = np.asarray(boxes, np.float32)
    B, R, _ = boxes.shape
    N = B * R

    idx, w4 = _compute_refs(boxes)
    arenas = [_build_arena(x2, x3, x4, x5, b) for b in range(B)]
    mask2 = (np.arange(64)[None, :] == (np.arange(128)[:, None] // 2)).astype(np.float32)

    in_maps = []
    for core in range(N_CORES):
        r0 = core * ROIS_PER_CORE
        par = _pack_core(idx[r0:r0 + ROIS_PER_CORE], w4[r0:r0 + ROIS_PER_CORE])
        in_maps.append({
            "arena": arenas[r0 // R],
            "par": par,
            "mask2": mask2,
        })

    nc = _get_program()
    global LAST_EXEC_NS
    if TRACE:
        _enable_ntff_hook()
        res = run_bass_kernel_spmd(nc, in_maps, list(range(N_CORES)), trace=True)
        LAST_EXEC_NS = res.exec_time_ns
    else:
        res = run_bass_kernel_spmd(nc, in_maps, list(range(N_CORES)))
    outs = []
    for core in range(N_CORES):
        o = res.results[core]["out"]                      # [6272, 256]
        o = o.reshape(ROIS_PER_CORE, BINS, C).transpose(0, 2, 1)
        outs.append(o.reshape(ROIS_PER_CORE, C, P_OUT, P_OUT))
    return np.concatenate(outs, axis=0)


# revision 7
# speedup vs baseline: 1.2348x; 1.2348x over previous
"""ROIPooler (FPN ROIAlign, P=7, ratio=2) Trainium2 Bass kernel.

Strategy (8 NeuronCores, data-parallel over ROIs):
- 1024 ROIs -> 128 per core; cores 0-3 work on image 0, cores 4-7 on image 1.
- Host precomputes, per ROI, its FPN level, the 196 bilinear sample windows
  (49 bins x 4 subsamples) and the 4 corner weights per window (validity,
  edge clamps and the 1/4 average folded in).
- Each image's feature pyramid is laid out host-side as a "row-pair" arena:
  entry (l, y, x) = [feat_l[y, x, :C], feat_l[y+1, x, :C]] fp32, so ONE
  contiguous 1024-element read (2 entries) covers all 4 bilinear corners
  of a subsample.
- Device: per K-tile of 128 refs, one indirect DMA gathers [128, 1024];
  4 masked matmuls (lhsT = static one-hot bin mask x per-ref corner weight,
  built on DVE) reduce refs into PSUM bins; PSUM groups of 128 bins are
  evacuated and DMA'd to a dense [6272, 256] output per core.
"""
import numpy as np

P_OUT = 7
RATIO = 2
SCALES = (0.25, 0.125, 0.0625, 0.03125)
LEVEL_DIMS = ((200, 336), (100, 168), (50, 84), (25, 42))
CANONICAL_BOX_SIZE = 224.0
CANONICAL_LEVEL = 4.0
EPS = 2.220446049250313e-16
C = 256
ENTRY = 2 * C                 # row-pair entry elems
WIN = 2 * ENTRY               # gather window elems (2 entries = 4 corners)
BINS = 49
REFS_PER_BIN = 4              # 2x2 subsamples
REFS_PER_ROI = BINS * REFS_PER_BIN       # 196
ROIS_PER_CORE = 128
N_CORES = 8
N_TILES = ROIS_PER_CORE * REFS_PER_ROI // 128    # 196 K-tiles per core
N_BINS_CORE = ROIS_PER_CORE * BINS               # 6272 = 49 groups of 128
T_CHUNK = 12                  # K-tiles per chunk (3 psum groups)
PLANE_PX = [h * w for h, w in LEVEL_DIMS]
PLANE_BASE_PX = [0]
for _px in PLANE_PX:
    PLANE_BASE_PX.append(PLANE_BASE_PX[-1] + _px)
ARENA_PX = PLANE_BASE_PX[-1]                     # 89250 entries
ARENA_ELEMS = ARENA_PX * ENTRY + WIN             # + pad tail

_CHUNKS = []
_t = 0
while _t < N_TILES:
    _CHUNKS.append(min(T_CHUNK, N_TILES - _t))
    _t += T_CHUNK
N_CHUNKS = len(_CHUNKS)                          # 17 (16x12 + 1x4)
PAR_COLS = 5 * T_CHUNK                           # idx + 4 weights per tile slot


GATHER_DTYPE = np.float32


def _build_arena(x2, x3, x4, x5, b):
    parts = []
    for x in (x2, x3, x4, x5):
        hwc = np.ascontiguousarray(np.transpose(x[b], (1, 2, 0))).astype(GATHER_DTYPE)
        H, W, _ = hwc.shape
        rp = np.zeros((H, W, 2, C), GATHER_DTYPE)
        rp[:, :, 0] = hwc
        rp[:-1, :, 1] = hwc[1:]
        parts.append(rp.reshape(-1))
    parts.append(np.zeros(WIN, GATHER_DTYPE))
    return np.concatenate(parts)


def _compute_refs(boxes):
    """-> idx [N, 196] int32 (arena element offsets), w4 [N, 196, 4] f32.
    Ref order per ROI: bin-major (by*7+bx), then j = sy*2 + sx.
    Window corners: [0:256]=(y,x) [256:512]=(y+1,x) [512:768]=(y,x+1) [768:1024]=(y+1,x+1)."""
    f32 = np.float32
    B, R, _ = boxes.shape
    N = B * R
    flat = boxes.reshape(N, 4).astype(f32)
    x1, y1, x2b, y2b = flat[:, 0], flat[:, 1], flat[:, 2], flat[:, 3]

    area = (x2b - x1) * (y2b - y1)
    sizes = np.sqrt(area, dtype=f32)
    lvl_f = np.floor(f32(CANONICAL_LEVEL)
                     + np.log2(sizes / f32(CANONICAL_BOX_SIZE) + f32(EPS), dtype=f32))
    lvl = np.clip(lvl_f, 2, 5).astype(np.int32) - 2

    scale = np.array(SCALES, f32)[lvl]
    H = np.array([d[0] for d in LEVEL_DIMS], np.int32)[lvl]
    W = np.array([d[1] for d in LEVEL_DIMS], np.int32)[lvl]
    base = np.array(PLANE_BASE_PX, np.int64)[:4][lvl]

    x1s = x1 * scale
    y1s = y1 * scale
    roi_w = np.maximum(x2b * scale - x1s, f32(1.0))
    roi_h = np.maximum(y2b * scale - y1s, f32(1.0))

    g = RATIO
    off = (np.arange(P_OUT, dtype=f32)[:, None]
           + (np.arange(g, dtype=f32)[None, :] + f32(0.5)) / f32(g)).reshape(-1)
    ys = y1s[:, None] + off[None, :] * (roi_h / f32(P_OUT))[:, None]
    xs = x1s[:, None] + off[None, :] * (roi_w / f32(P_OUT))[:, None]
    my = (ys >= -1.0) & (ys <= H[:, None])
    mx = (xs >= -1.0) & (xs <= W[:, None])

    def prep(t, L):
        t = np.maximum(t, f32(0.0))
        lo = np.floor(t).astype(np.int32)
        over = lo >= (L[:, None] - 1)
        lo = np.where(over, L[:, None] - 1, lo)
        t = np.where(over, lo.astype(f32), t)
        return lo, (t - lo).astype(f32)

    yl, fy = prep(ys, H)
    xl, fx = prep(xs, W)

    valid = (my[:, :, None] & mx[:, None, :]).astype(f32)     # [N,14,14]
    quarter = f32(1.0 / (g * g))

    YL = yl.reshape(N, P_OUT, g, 1, 1)
    FY = fy.reshape(N, P_OUT, g, 1, 1)
    XL = xl.reshape(N, 1, 1, P_OUT, g)
    FX = fx.reshape(N, 1, 1, P_OUT, g)
    V = valid.reshape(N, P_OUT, g, P_OUT, g)
    Wl = W.reshape(N, 1, 1, 1, 1).astype(np.int64)
    basel = base.reshape(N, 1, 1, 1, 1)

    ent = basel + YL.astype(np.int64) * Wl + XL.astype(np.int64)    # entry idx
    w00 = (1 - FY) * (1 - FX) * V * quarter
    w10 = FY * (1 - FX) * V * quarter
    w01 = (1 - FY) * FX * V * quarter
    w11 = FY * FX * V * quarter
    w4 = np.stack([w00, w10, w01, w11], axis=-1)              # [N,7,2,7,2,4]

    # (by, sy, bx, sx) -> (by, bx, sy, sx)
    ent = ent.transpose(0, 1, 3, 2, 4).reshape(N, REFS_PER_ROI)
    w4 = w4.transpose(0, 1, 3, 2, 4, 5).reshape(N, REFS_PER_ROI, 4).astype(f32)
    idx = (ent * ENTRY).astype(np.int32)
    return idx, w4


def _pack_core(idx_core, w4_core):
    """idx_core [128, 196], w4_core [128, 196, 4] -> par [N_CHUNKS, 128, PAR_COLS] int32.
    K-tile (q, r, h): partition p -> bin gb = 128q + 64r + (p//2), ref j = 2h + (p%2)."""
    idx_flat = idx_core.reshape(-1, REFS_PER_BIN)             # [6272 bins, 4]
    w_flat = w4_core.reshape(-1, REFS_PER_BIN, 4)
    tiles_idx = np.zeros((N_TILES, 128), np.int32)
    tiles_w = np.zeros((N_TILES, 4, 128), np.float32)
    p = np.arange(128)
    m = p // 2
    b2 = p % 2
    for q in range(N_BINS_CORE // 128):
        for r in range(2):
            for h in range(2):
                t = q * 4 + r * 2 + h
                gb = 128 * q + 64 * r + m
                j = 2 * h + b2
                tiles_idx[t] = idx_flat[gb, j]
                tiles_w[t] = w_flat[gb, j].T                  # [4, 128]
    par = np.zeros((N_CHUNKS, 128, PAR_COLS), np.int32)
    for c, tc in enumerate(_CHUNKS):
        t0 = c * T_CHUNK
        blk_i = tiles_idx[t0:t0 + tc]                         # [tc, 128]
        blk_w = tiles_w[t0:t0 + tc]                           # [tc, 4, 128]
        par[c, :, 0:tc] = blk_i.T
        for q4 in range(4):
            par[c, :, (1 + q4) * T_CHUNK:(1 + q4) * T_CHUNK + tc] = \
                blk_w[:, q4, :].T.view(np.int32)
    return par


_PROGRAM = None
TRACE = False
LAST_EXEC_NS = None


def _enable_ntff_hook():
    """Register the NTFF profile hook missing from this image's antenv."""
    import sys as _sys
    import types as _types
    try:
        from antenv.axon_hooks import get_axon_ntff_profile_hook  # noqa: F401
        return
    except ImportError:
        pass
    try:
        import trn_agent_boot.trn_boot as _tb
        _mod = _types.ModuleType("antenv.axon_hooks")
        _state = {"hook": _tb._ntff_profile_via_ctypes("/opt/axon/libaxon_pjrt.so")}
        _mod.set_axon_ntff_profile_hook = lambda h: _state.__setitem__("hook", h)
        _mod.get_axon_ntff_profile_hook = lambda: _state["hook"]
        _sys.modules["antenv.axon_hooks"] = _mod
    except Exception:
        pass


def _build_program():
    import concourse.bass as bass
    from concourse import mybir

    nc = bass.Bass()
    arena_ext = nc.declare_dram_parameter("arena", [ARENA_ELEMS], mybir.dt.float32, isOutput=False)
    par_ext = nc.declare_dram_parameter("par", [N_CHUNKS, 128, PAR_COLS], mybir.dt.int32, isOutput=False)
    mask_ext = nc.declare_dram_parameter("mask2", [128, 64], mybir.dt.float32, isOutput=False)
    out_ext = nc.declare_dram_parameter("out", [N_BINS_CORE, C], mybir.dt.float32, isOutput=True)

    T = T_CHUNK
    n_groups = [tc // 4 for tc in _CHUNKS]        # psum groups per chunk
    gd = np.cumsum([0] + n_groups)                # groups done after chunk c-1
    td = np.cumsum([0] + _CHUNKS)                 # tiles done
    NG = int(gd[-1])                              # 49

    with (
        nc.sbuf_tensor([128, 2 * PAR_COLS], mybir.dt.int32) as par_t,
        nc.sbuf_tensor([128, 64], mybir.dt.float32) as mask_t,
        nc.sbuf_tensor([128, 2 * T * 4 * 64], mybir.dt.float32) as W_t,
        nc.sbuf_tensor([128, 2 * T * WIN], mybir.dt.float32) as G_t,
        nc.sbuf_tensor([128, 2 * 3 * C], mybir.dt.float32) as stg_t,
        nc.psum_tensor([128, 4096], mybir.dt.float32) as ps,
        nc.semaphore("par_sem0") as par_sem0,
        nc.semaphore("par_sem1") as par_sem1,
        nc.semaphore("mask_sem") as mask_sem,
        nc.semaphore("gather_sem0") as gather_sem0,
        nc.semaphore("gather_sem1") as gather_sem1,
        nc.semaphore("w_sem") as w_sem,
        nc.semaphore("mm_sem") as mm_sem,
        nc.semaphore("evac_sem") as evac_sem,
        nc.semaphore("od_sem0") as od_sem0,
        nc.semaphore("od_sem1") as od_sem1,
        nc.Block() as block,
    ):
        par_sems = (par_sem0, par_sem1)
        gather_sems = (gather_sem0, gather_sem1)
        od_sems = (od_sem0, od_sem1)

        def par_count(cc):
            # value of par_sems[cc % 2] after par DMA of chunk cc completes
            return 16 * (cc // 2 + 1)

        def gather_count(cc):
            # value of gather_sems[cc % 2] after all gathers of chunk cc complete
            return 16 * sum(tcj for j, tcj in enumerate(_CHUNKS)
                            if j <= cc and j % 2 == cc % 2)

        def od_count(cc):
            # value of od_sems[cc % 2] after out-DMA of chunk cc completes
            return 16 * (cc // 2 + 1)

        def par_buf(c):
            return par_t[:, (c % 2) * PAR_COLS:(c % 2 + 1) * PAR_COLS]

        def w_buf(c):
            off = (c % 2) * T * 4 * 64
            return W_t[:, off:off + T * 4 * 64]

        def g_buf(c):
            off = (c % 2) * T * WIN
            return G_t[:, off:off + T * WIN]

        def stg_buf(c):
            off = (c % 2) * 3 * C
            return stg_t[:, off:off + 3 * C]

        @block.sync
        def _(sync):
            sync.dma_start(out=mask_t[:], in_=mask_ext[:]).then_inc(mask_sem, 16)

            def out_dma(sync, k):
                sync.wait_ge(evac_sem, int(gd[k + 1]))
                ng = n_groups[k]
                dst = out_ext[:].rearrange("(a p) c -> p a c", p=128)[
                    :, 3 * k:3 * k + ng, :]
                sync.dma_start(out=dst, in_=stg_buf(k)[:, :ng * C]).then_inc(
                    od_sems[k % 2], 16)

            for c in range(N_CHUNKS):
                if c >= 2:
                    # par buffer reuse: gather + W-build of c-2 finished
                    sync.wait_ge(gather_sems[c % 2], gather_count(c - 2))
                    sync.wait_ge(w_sem, c - 1)
                sync.dma_start(out=par_buf(c), in_=par_ext[c]).then_inc(
                    par_sems[c % 2], 16)
                if c >= 2:
                    out_dma(sync, c - 2)
            for k in (N_CHUNKS - 2, N_CHUNKS - 1):
                out_dma(sync, k)

        @block.gpsimd
        def _(g):
            for c, tc in enumerate(_CHUNKS):
                g.wait_ge(par_sems[c % 2], par_count(c))
                if c >= 2:
                    g.wait_ge(mm_sem, int(gd[c - 1]))   # G buffer reuse
                idxs = par_buf(c)
                gb = g_buf(c)
                for t in range(tc):
                    g.indirect_dma_start(
                        out=gb[:, t * WIN:(t + 1) * WIN],
                        out_offset=None,
                        in_=arena_ext[:].rearrange("(a b) -> a b", a=1),
                        in_offset=bass.IndirectOffsetOnAxis(ap=idxs[:, t:t + 1], axis=1),
                    ).then_inc(gather_sems[c % 2], 16)

        def _evacs(vector, k):
            ng = n_groups[k]
            if k >= 2:
                vector.wait_ge(od_sems[k % 2], od_count(k - 2))  # staging reuse
            for kg in range(ng):
                vector.wait_ge(mm_sem, int(gd[k]) + kg + 1)
                vector.tensor_copy(
                    out=stg_buf(k)[:, kg * C:(kg + 1) * C],
                    in_=ps[:, ((k % 2) * 4 + kg) * 512:((k % 2) * 4 + kg) * 512 + C],
                ).then_inc(evac_sem, 1)

        @block.vector
        def _(vector):
            vector.wait_ge(mask_sem, 16)
            for c, tc in enumerate(_CHUNKS):
                vector.wait_ge(par_sems[c % 2], par_count(c))
                if c >= 2:
                    vector.wait_ge(mm_sem, int(gd[c - 1]))  # W buffer reuse
                wsrc = par_buf(c)[:, T:5 * T].bitcast(mybir.dt.float32)
                # W[p, (t*4+q)*64 + m] = mask2[p, m] * w[p, q, t]
                vector.tensor_tensor(
                    out=w_buf(c)[:, :tc * 4 * 64].rearrange(
                        "p (t q m) -> p t q m", q=4, m=64),
                    in0=mask_t[:].rearrange("p (t q m) -> p t q m", t=1, q=1)
                        .to_broadcast([128, tc, 4, 64]),
                    in1=wsrc.rearrange("p (q t) -> p t q", q=4)[:, 0:tc, :]
                        .rearrange("p t (q m) -> p t q m", m=1)
                        .to_broadcast([128, tc, 4, 64]),
                    op=mybir.AluOpType.mult,
                ).then_inc(w_sem, 1)
                if c >= 1:
                    _evacs(vector, c - 1)
            _evacs(vector, N_CHUNKS - 1)

        @block.tensor
        def _(tensor):
            for c, tc in enumerate(_CHUNKS):
                # same-parity tiles gathered before this chunk
                prior = gather_count(c - 2) if c >= 2 else 0
                tensor.wait_ge(w_sem, c + 1)
                wb = w_buf(c)
                gb = g_buf(c)
                for kg in range(tc // 4):
                    # group kg needs only its own 4 K-tiles gathered
                    tensor.wait_ge(gather_sems[c % 2], prior + 16 * 4 * (kg + 1))
                    if c >= 2:
                        # psum slot (c%2, kg) free once evac (c-2, kg) done
                        tensor.wait_ge(evac_sem, int(gd[c - 2]) + kg + 1)
                    pcol = ((c % 2) * 4 + kg) * 512
                    for r in range(2):
                        rows = slice(64 * r, 64 * r + 64)
                        for h in range(2):
                            t = kg * 4 + r * 2 + h
                            for q4 in range(4):
                                mm = tensor.matmul(
                                    out=ps[rows, pcol:pcol + C],
                                    lhsT=wb[:, (t * 4 + q4) * 64:(t * 4 + q4 + 1) * 64],
                                    rhs=gb[:, t * WIN + q4 * C:t * WIN + (q4 + 1) * C],
                                    start=(h == 0 and q4 == 0),
                                    stop=(h == 1 and q4 == 3),
                                )
                                if r == 1 and h == 1 and q4 == 3:
                                    mm.then_inc(mm_sem, 1)
    return nc


def _get_program():
    global _PROGRAM
    if _PROGRAM is None:
        _PROGRAM = _build_program()
    return _PROGRAM


def kernel(x2, x3, x4, x5, boxes):
    from concourse.bass_utils import run_bass_kernel_spmd

    x2 = np.asarray(x2, np.float32)
    x3 = np.asarray(x3, np.float32)
    x4 = np.asarray(x4, np.float32)
    x5 = np.asarray(x5, np.float32)
    boxes = np.asarray(boxes, np.float32)
    B, R, _ = boxes.shape
    N = B * R

    idx, w4 = _compute_refs(boxes)
    arenas = [_build_arena(x2, x3, x4, x5, b) for b in range(B)]
    mask2 = (np.arange(64)[None, :] == (np.arange(128)[:, None] // 2)).astype(np.float32)

    in_maps = []
    for core in range(N_CORES):
        r0 = core * ROIS_PER_CORE
        par = _pack_core(idx[r0:r0 + ROIS_PER_CORE], w4[r0:r0 + ROIS_PER_CORE])
        in_maps.append({
            "arena": arenas[r0 // R],
            "par": par,
            "mask2": mask2,
        })

    nc = _get_program()
    global LAST_EXEC_NS
    if TRACE:
        _enable_ntff_hook()
        res = run_bass_kernel_spmd(nc, in_maps, list(range(N_CORES)), trace=True)
        LAST_EXEC_NS = res.exec_time_ns
    else:
        res = run_bass_kernel_spmd(nc, in_maps, list(range(N_CORES)))
    outs = []
    for core in range(N_CORES):
        o = res.results[core]["out"]                      # [6272, 256]
        o = o.reshape(ROIS_PER_CORE, BINS, C).transpose(0, 2, 1)
        outs.append(o.reshape(ROIS_PER_CORE, C, P_OUT, P_OUT))
    return np.concatenate(outs, axis=0)
